# revision 5
# baseline (speedup 1.0000x reference)
# BitStackLinear Trainium2 kernel (8-core column-parallel).
#
# reference computation:
#   sign  = unpack_bits(qweight) in {-1,+1}            [4, 4096, 4096]  (b, o, i)
#   w     = sum_b sign_b * (u_b @ vt_b)                [4096, 4096]     (o, i)
#   out   = x @ w.T                                    [4, 2048, 4096]
#
# Sharding: column-parallel over out_features (512 per core). x replicated.
#
# Per-core device program:
#   Phase F (formation of w.T [in_f, 512] in SBUF, fp16):
#     For each i-tile (128 rows of in_f):
#       - L_b.T[i, o] = vt_b.T @ u_b.T  via PE (K=16), psum f32 -> fp16 (ScalarE)
#       - sign masks m_b in {0, 0x8000} via DVE tensor_scalar(AND, SHL)
#         (host packs INVERTED bits, so m=0x8000 exactly when sign is -1)
#       - prod_b = L_b XOR m_b   (uint16 view: flips fp16 sign bit -> exact +-L)
#       - wT_tile = (p0 + p1) + (p2 + p3)   (fp16 adds)
#   Phase M: out[t, o] = sum_i xT[i, t].T @ wT[i, o]  (PE, fp32 psum, fp16 out)
#
# Host prep: transpose x to [in_f, tokens]; repack qweight bits into uint16
# words so that on-device bit l of word j covers output column o = 32*l + j
# (bit-plane-major -> unpacked masks land contiguous in o, no permutation).

import sys

import numpy as np

for p in ("/opt/trn_rl_repo", "/opt/pypackages"):
    if p not in sys.path:
        sys.path.insert(0, p)

import concourse.bacc as bacc
import concourse.bass as bass
import concourse.mybir as mybir
import concourse.tile as tile
from concourse.bass_utils import run_bass_kernel_spmd

W_BIT, OUT_F, IN_F, K = 4, 4096, 4096, 16
B, S = 4, 2048
T = B * S                      # 8192 tokens
NCORES = 8
OS = OUT_F // NCORES           # 512 out features per core
N_ITILES = IN_F // 128         # 32
N_TGROUPS = 8                  # token groups of 1024 (8 psum banks of 128)
TG = T // N_TGROUPS            # 1024 tokens per group

FP16 = mybir.dt.float16
F32 = mybir.dt.float32
U16 = mybir.dt.uint16
Alu = mybir.AluOpType

_cached = {}


def build_nc():
    nc = bacc.Bacc("TRN2", target_bir_lowering=False, debug=False,
                   num_devices=NCORES)
    xt_p = nc.dram_tensor("xt", [IN_F, T], FP16, kind="ExternalInput").ap()
    qp_p = nc.dram_tensor("qp", [IN_F, 128], U16, kind="ExternalInput").ap()
    ut_p = nc.dram_tensor("ut", [W_BIT, K, OS], FP16, kind="ExternalInput").ap()
    vt_p = nc.dram_tensor("vt4", [W_BIT, K, IN_F], FP16, kind="ExternalInput").ap()
    out_p = nc.dram_tensor("out", [T, OS], FP16, kind="ExternalOutput").ap()

    with tile.TileContext(nc) as tc:
        with (
            tc.tile_pool(name="const", bufs=1) as cpool,
            tc.tile_pool(name="wt", bufs=1) as wtpool,
        ):
            # resident operands
            vt_b = []
            ut_b = []
            for b in range(W_BIT):
                v = cpool.tile([K, IN_F], FP16, tag=f"vt{b}")
                nc.sync.dma_start(v[:], vt_p[b, :, :])
                vt_b.append(v)
                uu = cpool.tile([K, OS], FP16, tag=f"ut{b}")
                nc.sync.dma_start(uu[:], ut_p[b, :, :])
                ut_b.append(uu)

            wt = wtpool.tile([128, N_ITILES * OS], FP16, tag="wt")  # w.T, fp16

            # ---- Phase F: form w.T ----
            with (
                tc.tile_pool(name="fq", bufs=N_ITILES) as fq,
                tc.tile_pool(name="fpsL", bufs=8, space="PSUM") as fpsL,
                tc.tile_pool(name="fbits", bufs=2) as fb,
                tc.tile_pool(name="fl", bufs=8) as fl,
                tc.tile_pool(name="fa", bufs=4) as fa,
            ):
                for it in range(N_ITILES):
                    isl = slice(it * 128, (it + 1) * 128)
                    q = fq.tile([128, 128], U16, tag="q")
                    nc.sync.dma_start(q[:], qp_p[isl, :])

                    # sign masks in {0, 0x8000}, one tensor_scalar per bit l
                    masks = fb.tile([128, W_BIT * OS], U16, tag="masks")
                    q3 = q[:].rearrange("p (b j) -> p b j", b=W_BIT)
                    m4 = masks[:].rearrange("p (b l j) -> p b l j", b=W_BIT, l=16)
                    for l in range(16):
                        nc.vector.tensor_scalar(
                            m4[:, :, l, :], q3, 1 << l, 15 - l,
                            op0=Alu.bitwise_and, op1=Alu.logical_shift_left,
                        )

                    ls_b = []
                    for b in range(W_BIT):
                        pl = fpsL.tile([128, OS], F32, tag="L")
                        nc.tensor.matmul(
                            pl[:], vt_b[b][:, isl], ut_b[b][:], start=True, stop=True
                        )
                        ls = fl.tile([128, OS], FP16, tag="Ls")
                        nc.scalar.copy(ls[:], pl[:])
                        ls_b.append(ls)

                    # prod_b = L_b ^ m_b  (sign-bit flip)
                    prods = []
                    for b in range(W_BIT):
                        pr = fa.tile([128, OS], FP16, tag=f"pr{b}")
                        nc.vector.tensor_tensor(
                            pr[:].bitcast(U16),
                            ls_b[b][:].bitcast(U16),
                            masks[:, b * OS:(b + 1) * OS],
                            op=Alu.bitwise_xor,
                        )
                        prods.append(pr)
                    p01 = fa.tile([128, OS], FP16, tag="p01")
                    p23 = fa.tile([128, OS], FP16, tag="p23")
                    nc.vector.tensor_add(p01[:], prods[0][:], prods[1][:])
                    nc.vector.tensor_add(p23[:], prods[2][:], prods[3][:])
                    nc.vector.tensor_add(wt[:, it * OS:(it + 1) * OS], p01[:], p23[:])

            # ---- Phase M: main matmul ----
            with (
                tc.tile_pool(name="mx", bufs=4) as mx,
                tc.tile_pool(name="mps", bufs=8, space="PSUM") as mps,
                tc.tile_pool(name="mo", bufs=8) as mo,
            ):
                for g in range(N_TGROUPS):
                    accs = [
                        mps.tile([128, OS], F32, tag="acc", name=f"acc_{g}_{tt}")
                        for tt in range(8)
                    ]
                    for it in range(N_ITILES):
                        xs = mx.tile([128, TG], FP16, tag="x")
                        nc.sync.dma_start(
                            xs[:], xt_p[it * 128:(it + 1) * 128, g * TG:(g + 1) * TG]
                        )
                        for tt in range(8):
                            nc.tensor.matmul(
                                accs[tt][:],
                                xs[:, tt * 128:(tt + 1) * 128],
                                wt[:, it * OS:(it + 1) * OS],
                                start=(it == 0),
                                stop=(it == N_ITILES - 1),
                            )
                    for tt in range(8):
                        ot = mo.tile([128, OS], FP16, tag="o")
                        nc.scalar.copy(ot[:], accs[tt][:])
                        r0 = g * TG + tt * 128
                        nc.sync.dma_start(out_p[r0:r0 + 128, :], ot[:])
    nc.compile()
    return nc


def prep_inputs(x, qweight, u, vt):
    """Host-side shard prep. Returns per-core input maps."""
    x = np.asarray(x, dtype=np.float16)
    qweight = np.asarray(qweight)
    u = np.asarray(u, dtype=np.float16)
    vt = np.ascontiguousarray(np.asarray(vt, dtype=np.float16))

    xt = np.ascontiguousarray(x.reshape(T, IN_F).T)  # [IN_F, T]

    # unpack bits: (b, o, i); INVERT so mask=0x8000 <=> sign -1 (bit 0)
    bytes_ = qweight.astype(np.uint8)
    bits = np.unpackbits(bytes_.reshape(W_BIT, -1, 1), axis=2, bitorder="little")
    bits = bits.reshape(W_BIT, OUT_F, IN_F)
    inv = (1 - bits).astype(np.uint16)
    # word[c][i, b*32 + j] bit l = inv[b, 512c + 32l + j, i]
    bl = inv.reshape(W_BIT, NCORES, 16, 32, IN_F)  # [b, c, l, j, i]
    words = np.zeros((W_BIT, NCORES, 32, IN_F), np.uint16)
    for l in range(16):
        words |= bl[:, :, l, :, :] << np.uint16(l)
    qp_all = words.transpose(1, 3, 0, 2)  # [c, i, b, j]

    in_maps = []
    for c in range(NCORES):
        uc = u[:, c * OS:(c + 1) * OS, :]                  # [4, 512, 16]
        ut = np.ascontiguousarray(uc.transpose(0, 2, 1))  # [4, 16, 512]
        qp_c = np.ascontiguousarray(qp_all[c]).reshape(IN_F, 128)
        in_maps.append({"xt": xt, "qp": qp_c, "ut": ut, "vt4": vt})
    return in_maps


def kernel(x, qweight, u, vt, _trace=False):
    if "nc" not in _cached:
        _cached["nc"] = build_nc()
    nc = _cached["nc"]
    in_maps = prep_inputs(x, qweight, u, vt)
    res = run_bass_kernel_spmd(nc, in_maps, list(range(NCORES)), trace=_trace)
    _cached["last_result"] = res
    out = np.concatenate([res.results[c]["out"] for c in range(NCORES)], axis=1)
    return out.reshape(B, S, OUT_F).astype(np.float16)


# revision 9
# speedup vs baseline: 1.0739x; 1.0739x over previous
# BitStackLinear Trainium2 kernel (8-core column-parallel).
#
# reference computation:
#   sign  = unpack_bits(qweight) in {-1,+1}            [4, 4096, 4096]  (b, o, i)
#   w     = sum_b sign_b * (u_b @ vt_b)                [4096, 4096]     (o, i)
#   out   = x @ w.T                                    [4, 2048, 4096]
#
# Sharding: column-parallel over out_features (512 per core). x replicated.
#
# Per-core device program (single pass, formation pipelined into main loop):
#   For each i-tile (128 rows of in_f):
#     - L_b.T[i, o] = vt_b.T @ u_b.T via PE (K=16, psum f32) -> fp16 (ScalarE)
#     - sign masks m_b in {0, 0x8000} via DVE tensor_scalar(AND, SHL)
#       (host packs INVERTED bits, so m=0x8000 exactly when sign is -1)
#     - prod_b = L_b XOR m_b (uint16 view: flips fp16 sign bit -> exact +-L)
#     - wT_tile = (p0 + p1) + (p2 + p3)  (fp16 adds, batched wide)
#     - first token group's matmuls for this i-tile issue immediately
#   Then remaining token groups: out[t, o] = sum_i xT[i,t].T @ wT[i,o]
#   PSUM budget: 6 banks group-0 accumulators + 2 banks cycling L psums.
#
# Host prep: transpose x to [in_f, tokens]; repack qweight bits into uint16
# words so that on-device bit l of word j covers output column o = 32*l + j
# (bit-plane-major -> unpacked masks land contiguous in o, no permutation).

import sys

import numpy as np

for p in ("/opt/trn_rl_repo", "/opt/pypackages"):
    if p not in sys.path:
        sys.path.insert(0, p)

import concourse.bacc as bacc
import concourse.mybir as mybir
import concourse.tile as tile
from concourse.bass_utils import run_bass_kernel_spmd

W_BIT, OUT_F, IN_F, K = 4, 4096, 4096, 16
B, S = 4, 2048
T = B * S                      # 8192 tokens
NCORES = 8
OS = OUT_F // NCORES           # 512 out features per core
N_ITILES = IN_F // 128         # 32

# token groups: (start_token, n_ttiles). group 0 runs under formation with 6
# psum banks; the rest use 8; remainder group of 2 closes the books.
GROUPS = [(0, 6)] + [(768 + 1024 * g, 8) for g in range(7)] + [(7936, 2)]

FP16 = mybir.dt.float16
F32 = mybir.dt.float32
U16 = mybir.dt.uint16
Alu = mybir.AluOpType

_cached = {}


def build_nc():
    nc = bacc.Bacc("TRN2", target_bir_lowering=False, debug=False,
                   num_devices=NCORES)
    xt_p = nc.dram_tensor("xt", [IN_F, T], FP16, kind="ExternalInput").ap()
    qp_p = nc.dram_tensor("qp", [IN_F, 128], U16, kind="ExternalInput").ap()
    ut_p = nc.dram_tensor("ut", [W_BIT, K, OS], FP16, kind="ExternalInput").ap()
    vt_p = nc.dram_tensor("vt4", [W_BIT, K, IN_F], FP16, kind="ExternalInput").ap()
    out_p = nc.dram_tensor("out", [T, OS], FP16, kind="ExternalOutput").ap()

    with tile.TileContext(nc) as tc:
        with (
            tc.tile_pool(name="const", bufs=1) as cpool,
            tc.tile_pool(name="wt", bufs=1) as wtpool,
            tc.tile_pool(name="fq", bufs=N_ITILES) as fq,
            tc.tile_pool(name="fl", bufs=4) as fl,
            tc.tile_pool(name="fbits", bufs=4) as fb,
            tc.tile_pool(name="fa", bufs=4) as fa,
            tc.tile_pool(name="mx", bufs=6) as mx,
            tc.tile_pool(name="mps", bufs=8, space="PSUM") as mps,
            tc.tile_pool(name="mo", bufs=8) as mo,
        ):
            # resident operands
            vt_b = []
            ut_b = []
            for b in range(W_BIT):
                v = cpool.tile([K, IN_F], FP16, tag=f"vt{b}")
                nc.sync.dma_start(v[:], vt_p[b, :, :])
                vt_b.append(v)
                uu = cpool.tile([K, OS], FP16, tag=f"ut{b}")
                nc.sync.dma_start(uu[:], ut_p[b, :, :])
                ut_b.append(uu)

            # w.T tiles, one per i-tile (separate tiles so main-loop reads of
            # tile it' never falsely depend on formation writes of tile it)
            wts = [
                wtpool.tile([128, OS], FP16, tag=f"wt{it}", name=f"wt_{it}")
                for it in range(N_ITILES)
            ]

            def mm_group(gi, it):
                t0, ntt = GROUPS[gi]
                xs = mx.tile([128, ntt * 128], FP16, tag="x")
                nc.sync.dma_start(
                    xs[:], xt_p[it * 128:(it + 1) * 128, t0:t0 + ntt * 128]
                )
                for tt in range(ntt):
                    nc.tensor.matmul(
                        acc_tiles[tt][:],
                        xs[:, tt * 128:(tt + 1) * 128],
                        wts[it][:],
                        start=(it == 0),
                        stop=(it == N_ITILES - 1),
                    )

            def flush_group(gi):
                t0, ntt = GROUPS[gi]
                for tt in range(ntt):
                    ot = mo.tile([128, OS], FP16, tag="o")
                    nc.scalar.copy(ot[:], acc_tiles[tt][:])
                    r0 = t0 + tt * 128
                    nc.sync.dma_start(out_p[r0:r0 + 128, :], ot[:])

            # ---- formation pipelined with token group 0 ----
            acc_tiles = [
                mps.tile([128, OS], F32, tag="ps", name=f"acc_0_{tt}")
                for tt in range(GROUPS[0][1])
            ]
            for it in range(N_ITILES):
                isl = slice(it * 128, (it + 1) * 128)
                q = fq.tile([128, 128], U16, tag="q", name=f"q_{it}")
                nc.sync.dma_start(q[:], qp_p[isl, :])

                # low-rank psums -> fp16 (2 cycling psum slots, shared pool)
                ls = fl.tile([128, W_BIT * OS], FP16, tag="Ls")
                for b in range(W_BIT):
                    pl = mps.tile([128, OS], F32, tag="ps", name=f"pl_{it}_{b}")
                    nc.tensor.matmul(
                        pl[:], vt_b[b][:, isl], ut_b[b][:], start=True, stop=True
                    )
                    nc.scalar.copy(ls[:, b * OS:(b + 1) * OS], pl[:])

                # sign masks in {0, 0x8000}, one tensor_scalar per bit l
                masks = fb.tile([128, W_BIT * OS], U16, tag="masks")
                q3 = q[:].rearrange("p (b j) -> p b j", b=W_BIT)
                m4 = masks[:].rearrange("p (b l j) -> p b l j", b=W_BIT, l=16)
                for l in range(16):
                    nc.vector.tensor_scalar(
                        m4[:, :, l, :], q3, 1 << l, 15 - l,
                        op0=Alu.bitwise_and, op1=Alu.logical_shift_left,
                    )

                # prod_b = L_b ^ m_b for all 4 planes in one wide op
                prods = fa.tile([128, W_BIT * OS], FP16, tag="prods")
                nc.vector.tensor_tensor(
                    prods[:].bitcast(U16), ls[:].bitcast(U16), masks[:],
                    op=Alu.bitwise_xor,
                )
                p01 = fa.tile([128, 2 * OS], FP16, tag="p01")
                nc.vector.tensor_add(
                    p01[:], prods[:, 0:2 * OS], prods[:, 2 * OS:4 * OS]
                )
                nc.vector.tensor_add(wts[it][:], p01[:, 0:OS], p01[:, OS:2 * OS])

                mm_group(0, it)
            flush_group(0)

            # ---- remaining token groups ----
            for gi in range(1, len(GROUPS)):
                acc_tiles = [
                    mps.tile([128, OS], F32, tag="ps", name=f"acc_{gi}_{tt}")
                    for tt in range(GROUPS[gi][1])
                ]
                for it in range(N_ITILES):
                    mm_group(gi, it)
                flush_group(gi)
    nc.compile()
    return nc


def prep_inputs(x, qweight, u, vt):
    """Host-side shard prep. Returns per-core input maps."""
    x = np.asarray(x, dtype=np.float16)
    qweight = np.asarray(qweight)
    u = np.asarray(u, dtype=np.float16)
    vt = np.ascontiguousarray(np.asarray(vt, dtype=np.float16))

    xt = np.ascontiguousarray(x.reshape(T, IN_F).T)  # [IN_F, T]

    # unpack bits: (b, o, i); INVERT so mask=0x8000 <=> sign -1 (bit 0)
    bytes_ = qweight.astype(np.uint8)
    bits = np.unpackbits(bytes_.reshape(W_BIT, -1, 1), axis=2, bitorder="little")
    bits = bits.reshape(W_BIT, OUT_F, IN_F)
    inv = (1 - bits).astype(np.uint16)
    # word[c][i, b*32 + j] bit l = inv[b, 512c + 32l + j, i]
    bl = inv.reshape(W_BIT, NCORES, 16, 32, IN_F)  # [b, c, l, j, i]
    words = np.zeros((W_BIT, NCORES, 32, IN_F), np.uint16)
    for l in range(16):
        words |= bl[:, :, l, :, :] << np.uint16(l)
    qp_all = words.transpose(1, 3, 0, 2)  # [c, i, b, j]

    in_maps = []
    for c in range(NCORES):
        uc = u[:, c * OS:(c + 1) * OS, :]                 # [4, 512, 16]
        ut = np.ascontiguousarray(uc.transpose(0, 2, 1))  # [4, 16, 512]
        qp_c = np.ascontiguousarray(qp_all[c]).reshape(IN_F, 128)
        in_maps.append({"xt": xt, "qp": qp_c, "ut": ut, "vt4": vt})
    return in_maps


def kernel(x, qweight, u, vt, _trace=False):
    if "nc" not in _cached:
        _cached["nc"] = build_nc()
    nc = _cached["nc"]
    in_maps = prep_inputs(x, qweight, u, vt)
    res = run_bass_kernel_spmd(nc, in_maps, list(range(NCORES)), trace=_trace)
    _cached["last_result"] = res
    out = np.concatenate([res.results[c]["out"] for c in range(NCORES)], axis=1)
    return out.reshape(B, S, OUT_F).astype(np.float16)
